# revision 5
# baseline (speedup 1.0000x reference)
"""Trainium2 Bass kernel for nn_ClusterisedSelfAttentionNotLearnable.

Computes, for each point n (N=200000, data-parallel over 8 NeuronCores):
    enc    = posenc(X[n], 6 freqs)                      # [72]
    rgbc   = (enc @ L.T).reshape(256, 3)                # [256, 3]
    attn   = softmax(X[n, :3] @ cent.T)                 # [256]
    out[n] = attn @ rgbc                                # [3]

Device pipeline per 512-point chunk (features-on-partitions, points on
the free axis):
  host: range reduction!  x75 carries fp16 t = X/2pi (hi only, rows 0-2,
        for the cluster scores) and the CENTERED residues (2^f * t) mod 1
        in [-.5, .5] per (dim, freq) -- sin rows 3-38, pre-shifted cos
        rows 39-74 -- so the device-side angles are already inside the
        sin2pi table domain (strictly [-.5, .5]; the table has no
        internal range reduction) and no on-device mod/rint is needed.
        The softmax denominator's "+1" comes from a per-partition bias
        column (row 108 bias .25 -> sin2pi(.25) = 1), not a ones row.
  PE:   angle gather (sin + cos column halves), cluster scores,
        G = Lhat.T @ escore, and the final block reduction, which
        accumulates 16 chunks into one [64, NF] PSUM bank so the escape
        copy amortizes 16x.  (10 matmuls/chunk)
  DVE:  P = enc * G (two column halves), and the escape copy.
  ACT:  exp and sin2pi (shared exp_and_friends table -> no ACT table
        switches), plus a 1x1 tick after exp that decouples the
        single-buffered score bank from the next chunk.
  DMA:  the input slab streams through TWO queues in parallel (rows 0-37
        on the SP HWDGE queue, rows 38-74 on the gpsimd SWDGE queue) --
        a single queue is serviced by a single ~22 GB/s DMA engine, which
        was the previous version's end-to-end bottleneck.  Statics ride
        the scalar HWDGE queue (one-time), escapes ride gpsimd.
The softmax division happens on the host from the returned
[64, NG * NF] (3 numerators + denominator, 16 chunks per column group)
slab.
"""

import os as _os
import sys

sys.path.insert(0, "/opt/trn_rl_repo")

import ml_dtypes
import numpy as np

import concourse.bass as bass
import concourse.tile as tile
from concourse import mybir
from concourse.bass_utils import run_bass_kernel_spmd
from concourse.tile import TileContext, ScopedClock

# ---------------------------------------------------------------- constants
N = 200000
D = 6
C = 256
NFREQ = 6
NCORES = 8
NPC = N // NCORES           # 25000 points per core
NF = 512                    # points per chunk
CH = 49                     # chunks per core (49*512 = 25088 >= 25000)
NPAD = CH * NF              # 25088 padded points per core
GRP = 16                    # chunks per escape group (one [64, NF] PSUM bank)
NG = (CH + GRP - 1) // GRP  # escape groups per core
XR = 75                     # input slab rows
XSPLIT = int(_os.environ.get("KB_XSPLIT", 38))  # sync gets rows [0, XSPLIT)

F16 = mybir.dt.float16
BF16 = mybir.dt.bfloat16
F32 = mybir.dt.float32
NP_BF16 = ml_dtypes.bfloat16

_TWO_PI = 2.0 * np.pi

# ------------------------------------------------- harness compatibility patches


def _patch_tile_drain():
    """This walrus build rejects >2 sync waits on one instruction; spread the
    TileContext tail-drain waits across single-wait NOPs."""

    def _drain_and_barrier(self, tick_clock, wait_clock):
        nc = self.nc
        carrier = nc.sync.nop(nofuse=True)
        wait_clock.add_sem_waits(
            carrier.ins, ScopedClock({None: tick_clock.global_clock})
        )
        si = carrier.ins.sync_info
        waits = list(si.on_wait or []) if si is not None else []
        if len(waits) > 1:
            si.on_wait = waits[:1]
            for w in waits[1:]:
                extra = nc.sync.nop(nofuse=True)
                if extra.ins.sync_info is None:
                    extra.ins.sync_info = mybir.SyncInfo(on_wait=[w], on_update=[])
                else:
                    extra.ins.sync_info.on_wait = [w]
        nc.sync.drain()
        nc.all_engine_barrier()
        assert self.sems is not None
        popped = nc._tile_sem_poison_stack.pop()
        assert popped is self._sem_poison
        nc.clear_and_free_semaphores(list(self.sems.allocated().values()))
        nc.all_engine_barrier()

    TileContext._drain_and_barrier = _drain_and_barrier


def _split_excess_waits(nc, max_waits=1):
    """This walrus build accepts at most one sync wait per data instruction.
    Move excess waits onto injected same-engine NoOps placed directly before
    the over-subscribed instruction (waiting earlier on the same engine is
    semantically identical)."""
    ctr = 0
    for f in nc.m.functions:
        for bb in f.blocks:
            il = bb.instructions
            if not any(
                i.sync_info is not None
                and i.sync_info.on_wait
                and len(i.sync_info.on_wait) > max_waits
                for i in il
            ):
                continue
            new = []
            for inst in il:
                si = inst.sync_info
                waits = list(si.on_wait) if (si is not None and si.on_wait) else []
                if len(waits) > max_waits:
                    for w in waits[: len(waits) - max_waits]:
                        nop = mybir.InstNoOp(name=f"wsplit_nop_{ctr}", ins=[], outs=[])
                        ctr += 1
                        nop.engine = inst.engine
                        nop.sync_info = mybir.SyncInfo(on_wait=[w], on_update=[])
                        new.append(nop)
                    si.on_wait = waits[len(waits) - max_waits:]
                new.append(inst)
            bb.instructions = new


def _patch_sin2pi():
    """sin2pi (ACT func id 99) is not in the concourse enum but lives in the
    exp_and_friends table set. Emit Arctan as a marker and rewrite the
    serialized BIR."""
    if getattr(bass.Bass, "_sin2pi_patched", False):
        return
    orig = bass.Bass.to_json_bytes

    def to_json_bytes(self, *a, **k):
        return orig(self, *a, **k).replace(b'"Arctan"', b'"Sin2pi"')

    bass.Bass.to_json_bytes = to_json_bytes
    bass.Bass._sin2pi_patched = True


_patch_tile_drain()
_patch_sin2pi()

SIN2PI = mybir.ActivationFunctionType.Arctan  # rewritten to Sin2pi in the BIR

# ---------------------------------------------------------------- row maps
# Feature row k in [0, 108): k = dout*36 + din*6 + f.
_K = np.arange(108)
_DOUT = _K // 36
_DIN = (_K % 36) // 6
_F = _K % 6

# x75 row layout:
#   rows  0..2 : thi0..2 (fp16 turns, for the cluster scores -- matmul
#                operands must start at partition 0)
#   rows  3..38: fp16 centered sin residue (2^f t_din) mod 1, idx din*6+f
#   rows 39..74: fp16 centered cos residue (2^f t_din + .25) mod 1
#                (pre-shifting keeps BOTH sin2pi arg halves in [-.5, .5],
#                the table's strict domain)


def _build_static_arrays(linear_mappings, centroids):
    L = np.asarray(linear_mappings, dtype=np.float32)       # [768, 72]
    cent = np.asarray(centroids, dtype=np.float32)          # [256, 3]

    # Angle gather (residues are pre-reduced AND pre-shifted on the host,
    # so both weight sets are pure 0/1 row-gathers). Column 108: zero in
    # both halves; the cos half's sin2pi gets bias .25 on row 108, making
    # it the denominator's 1.
    wea = np.zeros((XR, 109), dtype=np.float16)
    wea[3 + _DIN * 6 + _F, _K] = 1.0
    web = np.zeros((XR, 109), dtype=np.float16)
    web[39 + _DIN * 6 + _F, _K] = 1.0

    # Scores: score_c = sum_d x_d * cent[c, d];  x = 2 pi t
    cent2pi = (cent * _TWO_PI).astype(np.float16)           # [256, 3]
    c6 = np.ascontiguousarray(cent2pi.T)                    # [3, 256]

    # Per-partition bias for the single [109, 2NF] sin2pi: row 108 gets
    # .25 so the (zero) angle there becomes sin2pi(.25) = 1. The sin
    # half's row 108 also sees the bias, but encA row 108 is never read.
    qb = np.zeros((109, 1), dtype=np.float32)
    qb[108, 0] = 0.25

    # Lhat: G_A[k] pairs with sin rows, G_B with cos rows, col 108 = denom.
    lhata = np.zeros((256, 108), dtype=np.float32)
    lhatb = np.zeros((256, 109), dtype=np.float32)
    ecol_sin = _DIN * 12 + _F
    ecol_cos = _DIN * 12 + 6 + _F
    for k in range(108):
        lhata[:, k] = L[3 * np.arange(C) + _DOUT[k], ecol_sin[k]]
        lhatb[:, k] = L[3 * np.arange(C) + _DOUT[k], ecol_cos[k]]
    lhatb[:, 108] = 1.0

    lp = np.zeros((128, 434), dtype=NP_BF16)
    lp[:, 0:108] = lhata[0:128]
    lp[:, 108:216] = lhata[128:256]
    lp[:, 216:325] = lhatb[0:128]
    lp[:, 325:434] = lhatb[128:256]

    # Escape-group reduction weights: chunk m of a group lands on output
    # rows 4m..4m+3 of the shared [64, NF] PSUM accumulator (matmul output
    # base partition must be 0, so the row placement is done by shifting
    # the weight columns; all other rows get zeros, which the PSUM
    # accumulation chain leaves untouched).
    rpg = np.zeros((109, 128 * 16), dtype=NP_BF16)
    for m in range(16):
        base = 128 * m
        rpg[_K, base + 4 * m + _DOUT] = 1.0           # RA block
        rpg[_K, base + 64 + 4 * m + _DOUT] = 1.0      # RB block
        rpg[108, base + 64 + 4 * m + 3] = 1.0         # denominator row

    return wea, web, c6, qb, lp, rpg


def _build_x75(X):
    """Per-core [75, NPAD] fp16 slabs: centered per-(dim, freq) sin and
    cos residues plus raw fp16 turns for the scores."""
    t = (np.asarray(X, dtype=np.float64) / _TWO_PI)          # [N, 6]
    x75 = np.zeros((NCORES, XR, NPAD), dtype=np.float16)
    r = t[None, :, :] * (2.0 ** np.arange(NFREQ))[:, None, None]  # [F, N, 6]
    rs = r - np.round(r)                                     # [-0.5, 0.5]
    rc = (r + 0.25) - np.round(r + 0.25)
    thi = t.astype(np.float16)
    for c in range(NCORES):
        seg = slice(c * NPC, (c + 1) * NPC)
        for d in range(3):
            x75[c, d, :NPC] = thi[seg, d]
        for d in range(D):
            for f in range(NFREQ):
                x75[c, 3 + d * 6 + f, :NPC] = rs[f, seg, d]
                x75[c, 39 + d * 6 + f, :NPC] = rc[f, seg, d]
    return x75


def _build_program():
    nc = bass.Bass()
    x75_h = nc.dram_tensor("x75", [XR, NPAD], F16, kind="ExternalInput")
    wea_h = nc.dram_tensor("wea", [XR, 109], F16, kind="ExternalInput")
    web_h = nc.dram_tensor("web", [XR, 109], F16, kind="ExternalInput")
    c6_h = nc.dram_tensor("c6", [3, 256], F16, kind="ExternalInput")
    qb_h = nc.dram_tensor("qb", [109, 1], F32, kind="ExternalInput")
    lp_h = nc.dram_tensor("lp", [128, 434], BF16, kind="ExternalInput")
    rp_h = nc.dram_tensor("rp", [109, 128 * GRP], BF16, kind="ExternalInput")
    # Row 4*(chunk%GRP)+d, col (group*NF + point) holds channel d (0..2 rgb
    # numerators, 3 = softmax denominator) of that chunk's point.
    o4_h = nc.dram_tensor("o4", [4 * GRP, NG * NF], F32, kind="ExternalOutput")

    EXP = mybir.ActivationFunctionType.Exp

    with TileContext(nc) as tc:
        with (
            tc.tile_pool(name="statics", bufs=1) as statics,
            tc.tile_pool(name="xin", bufs=int(_os.environ.get("KB_X", 3))) as xpool,
            tc.tile_pool(name="enc", bufs=int(_os.environ.get("KB_E", 3))) as encpool,
            tc.tile_pool(name="esc", bufs=int(_os.environ.get("KB_S", 3))) as escpool,
            tc.tile_pool(name="pprod", bufs=int(_os.environ.get("KB_P", 3))) as ppool,
            tc.tile_pool(name="rgbs", bufs=int(_os.environ.get("KB_R", 2))) as rspool,
            # PSUM bank budget (8 banks x 2KB/partition):
            #   ang 2 + sc 2 + gA 1 + gB 1 + rgbd 1 + warm 1 = 8
            tc.tile_pool(name="ang", bufs=int(_os.environ.get("KB_ANG", 1)), space="PSUM") as angpool,
            tc.tile_pool(name="sc", bufs=int(_os.environ.get("KB_SC", 1)), space="PSUM") as scpool,
            tc.tile_pool(name="gA", bufs=int(_os.environ.get("KB_GA", 1)), space="PSUM") as gApool,
            tc.tile_pool(name="gB", bufs=int(_os.environ.get("KB_GB", 1)), space="PSUM") as gBpool,
            tc.tile_pool(name="rgbd", bufs=int(_os.environ.get("KB_RG", 1)), space="PSUM") as rgbdpool,
            tc.tile_pool(name="warm", bufs=1, space="PSUM") as warmpool,
        ):
            wea_t = statics.tile([XR, 109], F16)
            web_t = statics.tile([XR, 109], F16)
            c6_t = statics.tile([3, 256], F16)
            qb_t = statics.tile([109, 1], F32)
            lp_t = statics.tile([128, 434], BF16)
            rp_t = statics.tile([109, 128 * GRP], BF16)
            # Statics ride the scalar HWDGE queue (one-time cost) so the
            # SP and gpsimd queues are clear for the input stream from the
            # first chunk on.
            nc.scalar.dma_start(out=c6_t[:], in_=c6_h[:])
            nc.scalar.dma_start(out=qb_t[:], in_=qb_h[:])
            nc.scalar.dma_start(out=wea_t[:], in_=wea_h[:])
            nc.scalar.dma_start(out=web_t[:], in_=web_h[:])
            nc.scalar.dma_start(out=lp_t[:], in_=lp_h[:])
            nc.scalar.dma_start(out=rp_t[:], in_=rp_h[:])

            # 1x1 scratch "tick" copies: cheap sem-incrementing ops the
            # scheduler can place freely; the ACT tick keeps the single-
            # buffered score bank from chaining the next chunk behind
            # sin2pi (pinning ticks to producer outputs measures worse --
            # the scheduler places free ticks better).
            dve_tk = statics.tile([1, 2], F16)
            act_tk = statics.tile([1, 2], F16)
            nc.vector.memset(dve_tk[:], 0.0)
            nc.vector.memset(act_tk[:], 0.0)

            def dve_tick():
                nc.vector.tensor_copy(out=dve_tk[0:1, 1:2], in_=dve_tk[0:1, 0:1])

            def act_tick():
                nc.scalar.copy(out=act_tk[0:1, 1:2], in_=act_tk[0:1, 0:1])

            # Dummy 1x1 exp: pre-loads the exp_and_friends ACT table during
            # the input-DMA ramp (it only depends on the memset scratch), so
            # the first real activation doesn't pay the ~1.3 us table load.
            nc.scalar.activation(
                out=act_tk[0:1, 1:2], in_=act_tk[0:1, 0:1],
                func=EXP, bias=0.0, scale=1.0,
            )

            # Dummy matmuls on scratch: pre-warm the PE p-state during the
            # input-DMA ramp so the first real matmuls run at full clock.
            warm_x = statics.tile([1, NF], F16)
            nc.vector.memset(warm_x[:], 0.0)
            warm_o = warmpool.tile([1, NF], F32)
            for _w in range(2):
                nc.tensor.matmul(
                    warm_o[0:1, 0:32], warm_x[0:1, 0:1], warm_x[0:1, 0:32],
                    start=True, stop=True,
                )

            # Software-pipelined: iteration k issues the early stages of
            # chunk k, the mid stages (G, P-mult) of chunk k-1, and the late
            # stages (reduction, escape) of chunk k-2.  Per-engine program
            # order then never forces a cheap early op of chunk k to queue
            # behind an expensive late-chain op of the same chunk, which
            # would serialize the whole loop at dependency-cycle latency.
            # The Tile wait machinery makes each cross-engine consumer wait
            # for completion of the instruction FOLLOWING its true producer
            # on the producer's engine.  The engine nops after each critical
            # producer make that "+1" instruction free, so consumers fire at
            # producer-completion instead of queueing behind the producer
            # engine's next (possibly late-blocking) real op.
            def early_sc(i):
                s = i * NF
                xt = xpool.tile([XR, NF], F16)
                # Two DMA queues in parallel: a single queue is one ~22
                # GB/s DMA engine, which cannot keep up with the compute.
                nc.sync.dma_start(out=xt[0:XSPLIT, :], in_=x75_h[0:XSPLIT, s:s + NF])
                nc.gpsimd.dma_start(out=xt[XSPLIT:XR, :], in_=x75_h[XSPLIT:XR, s:s + NF])

                sc = scpool.tile([128, 2 * NF], F32)
                nc.tensor.matmul(
                    sc[:, 0:NF], c6_t[:, 0:128], xt[0:3, :],
                    start=True, stop=True,
                )
                nc.tensor.matmul(
                    sc[:, NF:2 * NF], c6_t[:, 128:256], xt[0:3, :],
                    start=True, stop=True,
                )
                return xt, sc

            def mid(st):
                enc, esc = st
                gA = gApool.tile([108, NF], F32)
                gB = gBpool.tile([109, NF], F32)
                p = ppool.tile([109, 2 * NF], BF16)
                nc.tensor.matmul(
                    gA[:], lp_t[:, 0:108], esc[:, 0:NF],
                    start=True, stop=False,
                )
                nc.tensor.matmul(
                    gA[:], lp_t[:, 108:216], esc[:, NF:2 * NF],
                    start=False, stop=True,
                )
                nc.vector.tensor_mul(p[0:108, 0:NF], gA[:], enc[0:108, 0:NF])
                dve_tick()
                nc.tensor.matmul(
                    gB[:], lp_t[:, 216:325], esc[:, 0:NF],
                    start=True, stop=False,
                )
                nc.tensor.matmul(
                    gB[:], lp_t[:, 325:434], esc[:, NF:2 * NF],
                    start=False, stop=True,
                )
                nc.vector.tensor_mul(p[:, NF:2 * NF], gB[:], enc[:, NF:2 * NF])
                dve_tick()
                return p

            def early_act(i, xt):
                # Angles land already range-reduced (host residues): sin
                # half in [-.51, .51], cos half (pre-shifted residues) too.
                # Emitted AFTER the previous chunk's g matmuls so the DVE
                # mults start as early as possible, and right BEFORE the
                # ACT ops so sin2pi's wait resolves to the exact angB end.
                ang = angpool.tile([109, 2 * NF], F32)
                nc.tensor.matmul(
                    ang[0:109, 0:NF], wea_t[:], xt[:], start=True, stop=True
                )
                nc.tensor.matmul(
                    ang[0:109, NF:2 * NF], web_t[:], xt[:],
                    start=True, stop=True,
                )
                return ang

            def early_fin(i, sc, ang):
                # sin2pi first: the ang-bank read completes early in the
                # iteration, so the single-wait assigner can give the next
                # chunk's ang matmul a cheap dominating wait.  Row 108 of
                # the per-partition bias turns the cos half's zero angle
                # into sin2pi(.25) = 1, the softmax denominator.
                enc = encpool.tile([109, 2 * NF], F16)
                nc.scalar.activation(
                    out=enc[:], in_=ang[:], func=SIN2PI,
                    bias=qb_t[:], scale=1.0,
                )
                esc = escpool.tile([128, 2 * NF], BF16)
                nc.scalar.activation(
                    out=esc[:], in_=sc[:], func=EXP, bias=0.0, scale=1.0
                )
                act_tick()
                return enc, esc

            rgbdg = [None]   # current [64, NF] escape-group accumulator

            def late(p, i):
                m = i % GRP
                j = i // GRP
                if m == 0:
                    rgbdg[0] = rgbdpool.tile([4 * GRP, NF], F32, name="rgbdg")
                rgbd = rgbdg[0]
                last = (m == GRP - 1) or (i == CH - 1)
                nc.tensor.matmul(
                    rgbd[:],
                    rp_t[0:108, 128 * m:128 * m + 64], p[0:108, 0:NF],
                    start=(m == 0), stop=False,
                )
                nc.tensor.matmul(
                    rgbd[:],
                    rp_t[0:109, 128 * m + 64:128 * m + 128],
                    p[0:109, NF:2 * NF],
                    start=False, stop=last,
                )

                # One escape per GRP chunks: the copy's fixed cost is
                # amortized 16x (cost scales with columns, not partitions).
                # It rides on DVE, the least-loaded PSUM-capable engine;
                # the DMA rides gpsimd (tiny next to the input stream).
                if m == GRP - 1 or i == CH - 1:
                    rows = 4 * (m + 1)
                    rgbs = rspool.tile([4 * GRP, NF], F32)
                    nc.vector.tensor_copy(out=rgbs[0:rows, :], in_=rgbd[0:rows, :])
                    dve_tick()
                    nc.gpsimd.dma_start(
                        out=o4_h[0:rows, j * NF:(j + 1) * NF],
                        in_=rgbs[0:rows, :],
                    )

            prev_e = None   # (enc, esc) of chunk i-1
            prev_m = None   # p of chunk i-2
            prev_m2 = None  # p of chunk i-3
            for i in range(CH + 3):
                # late() leads: the reduction matmuls' inputs are THREE
                # chunks old, so at the head of the PE stream they never
                # wait and the engine stays dense.  (At depth two the DVE
                # products pA/pB -- which can only start after the G
                # matmuls that run at the END of the previous PE cycle --
                # arrive ~0.7 us late and stall both late matmuls every
                # chunk, which also keeps resetting the PE clock ramp.)
                if prev_m2 is not None:
                    late(prev_m2, i - 3)
                xs = early_sc(i) if i < CH else None
                m = mid(prev_e) if prev_e is not None else None
                if xs is not None:
                    ang = early_act(i, xs[0])
                    e = early_fin(i, xs[1], ang)
                else:
                    e = None
                prev_e, prev_m2, prev_m = e, prev_m, m

    _split_excess_waits(nc)
    return nc


_PROGRAM = None


def _get_program():
    global _PROGRAM
    if _PROGRAM is None:
        _PROGRAM = _build_program()
    return _PROGRAM


def kernel(X, linear_mappings, centroids, _want_trace=False):
    wea, web, c6, qb, lp, rp = _build_static_arrays(linear_mappings, centroids)
    x75 = _build_x75(X)

    nc = _get_program()
    in_maps = [
        {
            "x75": np.ascontiguousarray(x75[c]),
            "wea": wea, "web": web, "c6": c6, "qb": qb, "lp": lp, "rp": rp,
        }
        for c in range(NCORES)
    ]
    res = run_bass_kernel_spmd(
        nc, in_maps, core_ids=list(range(NCORES)), trace=_want_trace
    )

    out = np.empty((N, 3), dtype=np.float32)
    for c in range(NCORES):
        o4 = res.results[c]["o4"]                  # [4*GRP, NG*NF] f32
        # row 4*(chunk%GRP)+d, col (chunk//GRP)*NF + p -> chunk point p
        o4c = o4.reshape(GRP, 4, NG, NF)           # [m, d, j, p]
        o4c = o4c.transpose(2, 0, 3, 1)            # [j, m, p, d]
        flat = o4c.reshape(NG * GRP * NF, 4)[:NPC]  # per-core points
        out[c * NPC:(c + 1) * NPC, :] = flat[:, 0:3] / flat[:, 3:4]
    if _want_trace:
        return out, res
    return out


# revision 7
# speedup vs baseline: 1.0089x; 1.0089x over previous
"""Trainium2 Bass kernel for nn_ClusterisedSelfAttentionNotLearnable.

Computes, for each point n (N=200000, data-parallel over 8 NeuronCores):
    enc    = posenc(X[n], 6 freqs)                      # [72]
    rgbc   = (enc @ L.T).reshape(256, 3)                # [256, 3]
    attn   = softmax(X[n, :3] @ cent.T)                 # [256]
    out[n] = attn @ rgbc                                # [3]

Device pipeline per 512-point chunk (features-on-partitions, points on
the free axis):
  host: range reduction!  x75 carries fp16 t = X/2pi (hi only, rows 0-2,
        for the cluster scores) and the CENTERED residues (2^f * t) mod 1
        in [-.5, .5] per (dim, freq) -- sin rows 3-38, pre-shifted cos
        rows 39-74 -- so the device-side angles are already inside the
        sin2pi table domain (strictly [-.5, .5]; the table has no
        internal range reduction) and no on-device mod/rint is needed.
        The softmax denominator's "+1" comes from a per-partition bias
        column (row 108 bias .25 -> sin2pi(.25) = 1), not a ones row.
  PE:   angle gather (sin + cos column halves), cluster scores,
        G = Lhat.T @ escore, and the final block reduction, which
        accumulates 16 chunks into one [64, NF] PSUM bank so the escape
        copy amortizes 16x.  (10 matmuls/chunk)
  DVE:  P = enc * G (two column halves), and the escape copy.
  ACT:  exp and sin2pi (shared exp_and_friends table -> no ACT table
        switches), plus a 1x1 tick after exp that decouples the
        single-buffered score bank from the next chunk.
  DMA:  the input slab streams through TWO queues in parallel (rows 0-37
        on the SP HWDGE queue, rows 38-74 on the gpsimd SWDGE queue) --
        a single queue is serviced by a single ~22 GB/s DMA engine, which
        was the previous version's end-to-end bottleneck.  Statics ride
        the scalar HWDGE queue (one-time), escapes ride gpsimd.
The softmax division happens on the host from the returned
[64, NG * NF] (3 numerators + denominator, 16 chunks per column group)
slab.
"""

import os as _os
import sys

sys.path.insert(0, "/opt/trn_rl_repo")

import ml_dtypes
import numpy as np

import concourse.bass as bass
import concourse.tile as tile
from concourse import mybir
from concourse.bass_utils import run_bass_kernel_spmd
from concourse.tile import TileContext, ScopedClock

# ---------------------------------------------------------------- constants
N = 200000
D = 6
C = 256
NFREQ = 6
NCORES = 8
NPC = N // NCORES           # 25000 points per core
NF = 512                    # points per chunk
CH = 49                     # chunks per core (49*512 = 25088 >= 25000)
NPAD = CH * NF              # 25088 padded points per core
GRP = 16                    # chunks per escape group (one [64, NF] PSUM bank)
NG = (CH + GRP - 1) // GRP  # escape groups per core
XR = 75                     # input slab rows
XSPLIT = int(_os.environ.get("KB_XSPLIT", 38))  # sync gets rows [0, XSPLIT)

F16 = mybir.dt.float16
BF16 = mybir.dt.bfloat16
F32 = mybir.dt.float32
NP_BF16 = ml_dtypes.bfloat16

_TWO_PI = 2.0 * np.pi

# ------------------------------------------------- harness compatibility patches


def _patch_tile_drain():
    """This walrus build rejects >2 sync waits on one instruction; spread the
    TileContext tail-drain waits across single-wait NOPs."""

    def _drain_and_barrier(self, tick_clock, wait_clock):
        nc = self.nc
        carrier = nc.sync.nop(nofuse=True)
        wait_clock.add_sem_waits(
            carrier.ins, ScopedClock({None: tick_clock.global_clock})
        )
        si = carrier.ins.sync_info
        waits = list(si.on_wait or []) if si is not None else []
        if len(waits) > 1:
            si.on_wait = waits[:1]
            for w in waits[1:]:
                extra = nc.sync.nop(nofuse=True)
                if extra.ins.sync_info is None:
                    extra.ins.sync_info = mybir.SyncInfo(on_wait=[w], on_update=[])
                else:
                    extra.ins.sync_info.on_wait = [w]
        nc.sync.drain()
        nc.all_engine_barrier()
        assert self.sems is not None
        popped = nc._tile_sem_poison_stack.pop()
        assert popped is self._sem_poison
        nc.clear_and_free_semaphores(list(self.sems.allocated().values()))
        nc.all_engine_barrier()

    TileContext._drain_and_barrier = _drain_and_barrier


def _split_excess_waits(nc, max_waits=1):
    """This walrus build accepts at most one sync wait per data instruction.
    Move excess waits onto injected same-engine NoOps placed directly before
    the over-subscribed instruction (waiting earlier on the same engine is
    semantically identical)."""
    ctr = 0
    for f in nc.m.functions:
        for bb in f.blocks:
            il = bb.instructions
            if not any(
                i.sync_info is not None
                and i.sync_info.on_wait
                and len(i.sync_info.on_wait) > max_waits
                for i in il
            ):
                continue
            new = []
            for inst in il:
                si = inst.sync_info
                waits = list(si.on_wait) if (si is not None and si.on_wait) else []
                if len(waits) > max_waits:
                    for w in waits[: len(waits) - max_waits]:
                        nop = mybir.InstNoOp(name=f"wsplit_nop_{ctr}", ins=[], outs=[])
                        ctr += 1
                        nop.engine = inst.engine
                        nop.sync_info = mybir.SyncInfo(on_wait=[w], on_update=[])
                        new.append(nop)
                    si.on_wait = waits[len(waits) - max_waits:]
                new.append(inst)
            bb.instructions = new


def _patch_sin2pi():
    """sin2pi (ACT func id 99) is not in the concourse enum but lives in the
    exp_and_friends table set. Emit Arctan as a marker and rewrite the
    serialized BIR."""
    if getattr(bass.Bass, "_sin2pi_patched", False):
        return
    orig = bass.Bass.to_json_bytes

    def to_json_bytes(self, *a, **k):
        return orig(self, *a, **k).replace(b'"Arctan"', b'"Sin2pi"')

    bass.Bass.to_json_bytes = to_json_bytes
    bass.Bass._sin2pi_patched = True


_patch_tile_drain()
_patch_sin2pi()

SIN2PI = mybir.ActivationFunctionType.Arctan  # rewritten to Sin2pi in the BIR

# ---------------------------------------------------------------- row maps
# Feature row k in [0, 108): k = dout*36 + din*6 + f.
_K = np.arange(108)
_DOUT = _K // 36
_DIN = (_K % 36) // 6
_F = _K % 6

# x75 row layout:
#   rows  0..2 : thi0..2 (fp16 turns, for the cluster scores -- matmul
#                operands must start at partition 0)
#   rows  3..38: fp16 centered sin residue (2^f t_din) mod 1, idx din*6+f
#   rows 39..74: fp16 centered cos residue (2^f t_din + .25) mod 1
#                (pre-shifting keeps BOTH sin2pi arg halves in [-.5, .5],
#                the table's strict domain)


def _build_static_arrays(linear_mappings, centroids):
    L = np.asarray(linear_mappings, dtype=np.float32)       # [768, 72]
    cent = np.asarray(centroids, dtype=np.float32)          # [256, 3]

    # Angle gather (residues are pre-reduced AND pre-shifted on the host,
    # so both weight sets are pure 0/1 row-gathers). Column 108: zero in
    # both halves; the cos half's sin2pi gets bias .25 on row 108, making
    # it the denominator's 1.
    wea = np.zeros((XR, 109), dtype=np.float16)
    wea[3 + _DIN * 6 + _F, _K] = 1.0
    web = np.zeros((XR, 109), dtype=np.float16)
    web[39 + _DIN * 6 + _F, _K] = 1.0

    # Scores: score_c = sum_d x_d * cent[c, d];  x = 2 pi t
    cent2pi = (cent * _TWO_PI).astype(np.float16)           # [256, 3]
    c6 = np.ascontiguousarray(cent2pi.T)                    # [3, 256]

    # Per-partition bias for the single [109, 2NF] sin2pi: row 108 gets
    # .25 so the (zero) angle there becomes sin2pi(.25) = 1. The sin
    # half's row 108 also sees the bias, but encA row 108 is never read.
    qb = np.zeros((109, 1), dtype=np.float32)
    qb[108, 0] = 0.25

    # Lhat: G_A[k] pairs with sin rows, G_B with cos rows, col 108 = denom.
    lhata = np.zeros((256, 108), dtype=np.float32)
    lhatb = np.zeros((256, 109), dtype=np.float32)
    ecol_sin = _DIN * 12 + _F
    ecol_cos = _DIN * 12 + 6 + _F
    for k in range(108):
        lhata[:, k] = L[3 * np.arange(C) + _DOUT[k], ecol_sin[k]]
        lhatb[:, k] = L[3 * np.arange(C) + _DOUT[k], ecol_cos[k]]
    lhatb[:, 108] = 1.0

    lp = np.zeros((128, 434), dtype=NP_BF16)
    lp[:, 0:108] = lhata[0:128]
    lp[:, 108:216] = lhata[128:256]
    lp[:, 216:325] = lhatb[0:128]
    lp[:, 325:434] = lhatb[128:256]

    # Escape-group reduction weights: chunk m of a group lands on output
    # rows 4m..4m+3 of the shared [64, NF] PSUM accumulator (matmul output
    # base partition must be 0, so the row placement is done by shifting
    # the weight columns; all other rows get zeros, which the PSUM
    # accumulation chain leaves untouched).
    rpg = np.zeros((109, 128 * 16), dtype=NP_BF16)
    for m in range(16):
        base = 128 * m
        rpg[_K, base + 4 * m + _DOUT] = 1.0           # RA block
        rpg[_K, base + 64 + 4 * m + _DOUT] = 1.0      # RB block
        rpg[108, base + 64 + 4 * m + 3] = 1.0         # denominator row

    return wea, web, c6, qb, lp, rpg


def _build_x75(X):
    """Per-core [75, NPAD] fp16 slabs: centered per-(dim, freq) sin and
    cos residues plus raw fp16 turns for the scores."""
    t = (np.asarray(X, dtype=np.float64) / _TWO_PI)          # [N, 6]
    x75 = np.zeros((NCORES, XR, NPAD), dtype=np.float16)
    r = t[None, :, :] * (2.0 ** np.arange(NFREQ))[:, None, None]  # [F, N, 6]
    rs = r - np.round(r)                                     # [-0.5, 0.5]
    rc = (r + 0.25) - np.round(r + 0.25)
    thi = t.astype(np.float16)
    for c in range(NCORES):
        seg = slice(c * NPC, (c + 1) * NPC)
        for d in range(3):
            x75[c, d, :NPC] = thi[seg, d]
        for d in range(D):
            for f in range(NFREQ):
                x75[c, 3 + d * 6 + f, :NPC] = rs[f, seg, d]
                x75[c, 39 + d * 6 + f, :NPC] = rc[f, seg, d]
    return x75


def _build_program():
    nc = bass.Bass()
    x75_h = nc.dram_tensor("x75", [XR, NPAD], F16, kind="ExternalInput")
    wea_h = nc.dram_tensor("wea", [XR, 109], F16, kind="ExternalInput")
    web_h = nc.dram_tensor("web", [XR, 109], F16, kind="ExternalInput")
    c6_h = nc.dram_tensor("c6", [3, 256], F16, kind="ExternalInput")
    qb_h = nc.dram_tensor("qb", [109, 1], F32, kind="ExternalInput")
    lp_h = nc.dram_tensor("lp", [128, 434], BF16, kind="ExternalInput")
    rp_h = nc.dram_tensor("rp", [109, 128 * GRP], BF16, kind="ExternalInput")
    # Row 4*(chunk%GRP)+d, col (group*NF + point) holds channel d (0..2 rgb
    # numerators, 3 = softmax denominator) of that chunk's point.
    o4_h = nc.dram_tensor("o4", [4 * GRP, NG * NF], F32, kind="ExternalOutput")

    EXP = mybir.ActivationFunctionType.Exp

    with TileContext(nc) as tc:
        with (
            tc.tile_pool(name="statics", bufs=1) as statics,
            tc.tile_pool(name="xin", bufs=int(_os.environ.get("KB_X", 4))) as xpool,
            tc.tile_pool(name="enc", bufs=int(_os.environ.get("KB_E", 3))) as encpool,
            tc.tile_pool(name="esc", bufs=int(_os.environ.get("KB_S", 3))) as escpool,
            tc.tile_pool(name="pprod", bufs=int(_os.environ.get("KB_P", 3))) as ppool,
            tc.tile_pool(name="rgbs", bufs=int(_os.environ.get("KB_R", 2))) as rspool,
            # PSUM bank budget (8 banks x 2KB/partition):
            #   ang 2 + sc 2 + gA 1 + gB 1 + rgbd 1 + warm 1 = 8
            tc.tile_pool(name="ang", bufs=int(_os.environ.get("KB_ANG", 1)), space="PSUM") as angpool,
            tc.tile_pool(name="sc", bufs=int(_os.environ.get("KB_SC", 1)), space="PSUM") as scpool,
            tc.tile_pool(name="gA", bufs=int(_os.environ.get("KB_GA", 1)), space="PSUM") as gApool,
            tc.tile_pool(name="gB", bufs=int(_os.environ.get("KB_GB", 1)), space="PSUM") as gBpool,
            tc.tile_pool(name="rgbd", bufs=int(_os.environ.get("KB_RG", 1)), space="PSUM") as rgbdpool,
            tc.tile_pool(name="warm", bufs=1, space="PSUM") as warmpool,
        ):
            wea_t = statics.tile([XR, 109], F16)
            web_t = statics.tile([XR, 109], F16)
            c6_t = statics.tile([3, 256], F16)
            qb_t = statics.tile([109, 1], F32)
            lp_t = statics.tile([128, 434], BF16)
            # rp is 446KB -- by far the biggest static.  It goes through a
            # single ~22 GB/s DMA engine (~20us), and late(0) only needs
            # its first column group, so it is split into four separately
            # tracked tiles, ordered so early groups land first.  (A single
            # tile written by four DMAs would make late(0) wait for all
            # four -- tile dependency tracking is per-tile.)
            rp_ts = [statics.tile([109, 128 * 4], BF16, name=f"rp{j}")
                     for j in range(4)]
            # Statics ride the scalar HWDGE queue (one-time cost) so the
            # SP and gpsimd queues are clear for the input stream from the
            # first chunk on.  Order: everything the first chunks need
            # comes before the rp tail.
            nc.scalar.dma_start(out=wea_t[:], in_=wea_h[:])
            nc.scalar.dma_start(out=web_t[:], in_=web_h[:])
            nc.scalar.dma_start(out=c6_t[:], in_=c6_h[:])
            nc.scalar.dma_start(out=qb_t[:], in_=qb_h[:])
            nc.scalar.dma_start(out=lp_t[:], in_=lp_h[:])
            for j in range(4):
                nc.scalar.dma_start(
                    out=rp_ts[j][:], in_=rp_h[:, 512 * j:512 * (j + 1)]
                )

            # 1x1 scratch "tick" copies: cheap sem-incrementing ops the
            # scheduler can place freely; the ACT tick keeps the single-
            # buffered score bank from chaining the next chunk behind
            # sin2pi (pinning ticks to producer outputs measures worse --
            # the scheduler places free ticks better).
            dve_tk = statics.tile([1, 2], F16)
            act_tk = statics.tile([1, 2], F16)
            nc.vector.memset(dve_tk[:], 0.0)
            nc.vector.memset(act_tk[:], 0.0)

            def dve_tick():
                nc.vector.tensor_copy(out=dve_tk[0:1, 1:2], in_=dve_tk[0:1, 0:1])

            def act_tick():
                nc.scalar.copy(out=act_tk[0:1, 1:2], in_=act_tk[0:1, 0:1])

            # Dummy 1x1 exp: pre-loads the exp_and_friends ACT table during
            # the input-DMA ramp (it only depends on the memset scratch), so
            # the first real activation doesn't pay the ~1.3 us table load.
            nc.scalar.activation(
                out=act_tk[0:1, 1:2], in_=act_tk[0:1, 0:1],
                func=EXP, bias=0.0, scale=1.0,
            )

            # Dummy matmuls on scratch: pre-warm the PE p-state during the
            # input-DMA ramp so the first real matmuls run at full clock.
            warm_x = statics.tile([1, NF], F16)
            nc.vector.memset(warm_x[:], 0.0)
            warm_o = warmpool.tile([1, NF], F32)
            for _w in range(2):
                nc.tensor.matmul(
                    warm_o[0:1, 0:32], warm_x[0:1, 0:1], warm_x[0:1, 0:32],
                    start=True, stop=True,
                )

            # Software-pipelined: iteration k issues the early stages of
            # chunk k, the mid stages (G, P-mult) of chunk k-1, and the late
            # stages (reduction, escape) of chunk k-2.  Per-engine program
            # order then never forces a cheap early op of chunk k to queue
            # behind an expensive late-chain op of the same chunk, which
            # would serialize the whole loop at dependency-cycle latency.
            # The Tile wait machinery makes each cross-engine consumer wait
            # for completion of the instruction FOLLOWING its true producer
            # on the producer's engine.  The engine nops after each critical
            # producer make that "+1" instruction free, so consumers fire at
            # producer-completion instead of queueing behind the producer
            # engine's next (possibly late-blocking) real op.
            def early_sc(i):
                s = i * NF
                xt = xpool.tile([XR, NF], F16)
                # Two DMA queues in parallel: a single queue is one ~22
                # GB/s DMA engine, which cannot keep up with the compute.
                nc.sync.dma_start(out=xt[0:XSPLIT, :], in_=x75_h[0:XSPLIT, s:s + NF])
                nc.gpsimd.dma_start(out=xt[XSPLIT:XR, :], in_=x75_h[XSPLIT:XR, s:s + NF])

                sc = scpool.tile([128, 2 * NF], F32)
                nc.tensor.matmul(
                    sc[:, 0:NF], c6_t[:, 0:128], xt[0:3, :],
                    start=True, stop=True,
                )
                nc.tensor.matmul(
                    sc[:, NF:2 * NF], c6_t[:, 128:256], xt[0:3, :],
                    start=True, stop=True,
                )
                return xt, sc

            def mid(st):
                enc, esc = st
                gA = gApool.tile([108, NF], F32)
                gB = gBpool.tile([109, NF], F32)
                p = ppool.tile([109, 2 * NF], BF16)
                nc.tensor.matmul(
                    gA[:], lp_t[:, 0:108], esc[:, 0:NF],
                    start=True, stop=False,
                )
                nc.tensor.matmul(
                    gA[:], lp_t[:, 108:216], esc[:, NF:2 * NF],
                    start=False, stop=True,
                )
                nc.vector.tensor_mul(p[0:108, 0:NF], gA[:], enc[0:108, 0:NF])
                dve_tick()
                nc.tensor.matmul(
                    gB[:], lp_t[:, 216:325], esc[:, 0:NF],
                    start=True, stop=False,
                )
                nc.tensor.matmul(
                    gB[:], lp_t[:, 325:434], esc[:, NF:2 * NF],
                    start=False, stop=True,
                )
                nc.vector.tensor_mul(p[:, NF:2 * NF], gB[:], enc[:, NF:2 * NF])
                dve_tick()
                return p

            def early_act(i, xt):
                # Angles land already range-reduced (host residues): sin
                # half in [-.51, .51], cos half (pre-shifted residues) too.
                # Emitted AFTER the previous chunk's g matmuls so the DVE
                # mults start as early as possible, and right BEFORE the
                # ACT ops so sin2pi's wait resolves to the exact angB end.
                ang = angpool.tile([109, 2 * NF], F32)
                nc.tensor.matmul(
                    ang[0:109, 0:NF], wea_t[:], xt[:], start=True, stop=True
                )
                nc.tensor.matmul(
                    ang[0:109, NF:2 * NF], web_t[:], xt[:],
                    start=True, stop=True,
                )
                return ang

            def early_fin(i, sc, ang):
                # sin2pi first: the ang-bank read completes early in the
                # iteration, so the single-wait assigner can give the next
                # chunk's ang matmul a cheap dominating wait.  Row 108 of
                # the per-partition bias turns the cos half's zero angle
                # into sin2pi(.25) = 1, the softmax denominator.
                enc = encpool.tile([109, 2 * NF], F16)
                nc.scalar.activation(
                    out=enc[:], in_=ang[:], func=SIN2PI,
                    bias=qb_t[:], scale=1.0,
                )
                act_tick()
                esc = escpool.tile([128, 2 * NF], BF16)
                nc.scalar.activation(
                    out=esc[:], in_=sc[:], func=EXP, bias=0.0, scale=1.0
                )
                act_tick()
                return enc, esc

            rgbdg = [None]   # current [64, NF] escape-group accumulator

            def late(p, i):
                m = i % GRP
                j = i // GRP
                if m == 0:
                    rgbdg[0] = rgbdpool.tile([4 * GRP, NF], F32, name="rgbdg")
                rgbd = rgbdg[0]
                last = (m == GRP - 1) or (i == CH - 1)
                rp_t = rp_ts[m // 4]
                mm = m % 4
                nc.tensor.matmul(
                    rgbd[:],
                    rp_t[0:108, 128 * mm:128 * mm + 64], p[0:108, 0:NF],
                    start=(m == 0), stop=False,
                )
                nc.tensor.matmul(
                    rgbd[:],
                    rp_t[0:109, 128 * mm + 64:128 * mm + 128],
                    p[0:109, NF:2 * NF],
                    start=False, stop=last,
                )

                # One escape per GRP chunks: the copy's fixed cost is
                # amortized 16x (cost scales with columns, not partitions).
                # It rides on DVE, the least-loaded PSUM-capable engine;
                # the DMA rides gpsimd (tiny next to the input stream).
                if m == GRP - 1 or i == CH - 1:
                    rows = 4 * (m + 1)
                    rgbs = rspool.tile([4 * GRP, NF], F32)
                    nc.vector.tensor_copy(out=rgbs[0:rows, :], in_=rgbd[0:rows, :])
                    dve_tick()
                    nc.gpsimd.dma_start(
                        out=o4_h[0:rows, j * NF:(j + 1) * NF],
                        in_=rgbs[0:rows, :],
                    )

            prev_e = None   # (enc, esc) of chunk i-1
            prev_m = None   # p of chunk i-2
            prev_m2 = None  # p of chunk i-3
            for i in range(CH + 3):
                # late() leads: the reduction matmuls' inputs are THREE
                # chunks old, so at the head of the PE stream they never
                # wait and the engine stays dense.  (At depth two the DVE
                # products pA/pB -- which can only start after the G
                # matmuls that run at the END of the previous PE cycle --
                # arrive ~0.7 us late and stall both late matmuls every
                # chunk, which also keeps resetting the PE clock ramp.)
                if prev_m2 is not None:
                    late(prev_m2, i - 3)
                xs = early_sc(i) if i < CH else None
                m = mid(prev_e) if prev_e is not None else None
                if xs is not None:
                    ang = early_act(i, xs[0])
                    e = early_fin(i, xs[1], ang)
                else:
                    e = None
                prev_e, prev_m2, prev_m = e, prev_m, m

    _split_excess_waits(nc)
    return nc


_PROGRAM = None


def _get_program():
    global _PROGRAM
    if _PROGRAM is None:
        _PROGRAM = _build_program()
    return _PROGRAM


def kernel(X, linear_mappings, centroids, _want_trace=False):
    wea, web, c6, qb, lp, rp = _build_static_arrays(linear_mappings, centroids)
    x75 = _build_x75(X)

    nc = _get_program()
    in_maps = [
        {
            "x75": np.ascontiguousarray(x75[c]),
            "wea": wea, "web": web, "c6": c6, "qb": qb, "lp": lp, "rp": rp,
        }
        for c in range(NCORES)
    ]
    res = run_bass_kernel_spmd(
        nc, in_maps, core_ids=list(range(NCORES)), trace=_want_trace
    )

    out = np.empty((N, 3), dtype=np.float32)
    for c in range(NCORES):
        o4 = res.results[c]["o4"]                  # [4*GRP, NG*NF] f32
        # row 4*(chunk%GRP)+d, col (chunk//GRP)*NF + p -> chunk point p
        o4c = o4.reshape(GRP, 4, NG, NF)           # [m, d, j, p]
        o4c = o4c.transpose(2, 0, 3, 1)            # [j, m, p, d]
        flat = o4c.reshape(NG * GRP * NF, 4)[:NPC]  # per-core points
        out[c * NPC:(c + 1) * NPC, :] = flat[:, 0:3] / flat[:, 3:4]
    if _want_trace:
        return out, res
    return out


# revision 17
# speedup vs baseline: 1.3520x; 1.3401x over previous
"""Trainium2 Bass kernel for nn_ClusterisedSelfAttentionNotLearnable.

Computes, for each point n (N=200000, data-parallel over 8 NeuronCores):
    enc    = posenc(X[n], 6 freqs)                      # [72]
    rgbc   = (enc @ L.T).reshape(256, 3)                # [256, 3]
    attn   = softmax(X[n, :3] @ cent.T)                 # [256]
    out[n] = attn @ rgbc                                # [3]

Device pipeline per 512-point chunk (features-on-partitions, points on
the free axis):
  host: range reduction!  x75 carries fp16 t = X/2pi (hi only, rows 0-2,
        for the cluster scores) and the CENTERED residues (2^f * t) mod 1
        in [-.5, .5] per (dim, freq) -- sin rows 3-38, pre-shifted cos
        rows 39-74 -- so the device-side angles are already inside the
        sin2pi table domain (strictly [-.5, .5]; the table has no
        internal range reduction) and no on-device mod/rint is needed.
        The softmax denominator's "+1" comes from a per-partition bias
        column (row 108 bias .25 -> sin2pi(.25) = 1), not a ones row.
  PE:   angle gather (sin + cos column halves), cluster scores,
        G = Lhat.T @ escore, and the final block reduction, which
        accumulates 16 chunks into one [64, NF] PSUM bank so the escape
        copy amortizes 16x.  (10 matmuls/chunk)
  DVE:  P = enc * G (two column halves), and the escape copy.
  ACT:  exp and sin2pi (shared exp_and_friends table -> no ACT table
        switches), plus a 1x1 tick after exp that decouples the
        single-buffered score bank from the next chunk.
  DMA:  the input slab streams through TWO queues in parallel (rows 0-37
        on the SP HWDGE queue, rows 38-74 on the gpsimd SWDGE queue) --
        a single queue is serviced by a single ~22 GB/s DMA engine, which
        was the previous version's end-to-end bottleneck.  Statics ride
        the scalar HWDGE queue (one-time), escapes ride gpsimd.
The softmax division happens on the host from the returned
[64, NG * NF] (3 numerators + denominator, 16 chunks per column group)
slab.
"""

import os as _os
import sys

sys.path.insert(0, "/opt/trn_rl_repo")

import ml_dtypes
import numpy as np

import concourse.bass as bass
import concourse.tile as tile
from concourse import mybir
from concourse.bass_utils import run_bass_kernel_spmd
from concourse.tile import TileContext, ScopedClock

# ---------------------------------------------------------------- constants
N = 200000
D = 6
C = 256
NFREQ = 6
NCORES = 8
NPC = N // NCORES           # 25000 points per core
NF = 512                    # points per chunk
CH = 49                     # chunks per core (49*512 = 25088 >= 25000)
NPAD = CH * NF              # 25088 padded points per core
GRP = 16                    # chunks per escape group (one [64, NF] PSUM bank)
NG = (CH + GRP - 1) // GRP  # escape groups per core
XR = 75                     # input slab rows
XSPLIT = int(_os.environ.get("KB_XSPLIT", 38))  # sync gets rows [0, XSPLIT)

F16 = mybir.dt.float16
BF16 = mybir.dt.bfloat16
F32 = mybir.dt.float32
NP_BF16 = ml_dtypes.bfloat16

_TWO_PI = 2.0 * np.pi

# ------------------------------------------------- harness compatibility patches


def _patch_tile_drain():
    """This walrus build rejects >2 sync waits on one instruction; spread the
    TileContext tail-drain waits across single-wait NOPs."""

    def _drain_and_barrier(self, tick_clock, wait_clock):
        nc = self.nc
        carrier = nc.sync.nop(nofuse=True)
        wait_clock.add_sem_waits(
            carrier.ins, ScopedClock({None: tick_clock.global_clock})
        )
        si = carrier.ins.sync_info
        waits = list(si.on_wait or []) if si is not None else []
        if len(waits) > 1:
            si.on_wait = waits[:1]
            for w in waits[1:]:
                extra = nc.sync.nop(nofuse=True)
                if extra.ins.sync_info is None:
                    extra.ins.sync_info = mybir.SyncInfo(on_wait=[w], on_update=[])
                else:
                    extra.ins.sync_info.on_wait = [w]
        nc.sync.drain()
        nc.all_engine_barrier()
        assert self.sems is not None
        popped = nc._tile_sem_poison_stack.pop()
        assert popped is self._sem_poison
        nc.clear_and_free_semaphores(list(self.sems.allocated().values()))
        nc.all_engine_barrier()

    TileContext._drain_and_barrier = _drain_and_barrier


def _split_excess_waits(nc, max_waits=1):
    """This walrus build accepts at most one sync wait per data instruction.
    Move excess waits onto injected same-engine NoOps placed directly before
    the over-subscribed instruction (waiting earlier on the same engine is
    semantically identical)."""
    ctr = 0
    for f in nc.m.functions:
        for bb in f.blocks:
            il = bb.instructions
            if not any(
                i.sync_info is not None
                and i.sync_info.on_wait
                and len(i.sync_info.on_wait) > max_waits
                for i in il
            ):
                continue
            new = []
            for inst in il:
                si = inst.sync_info
                waits = list(si.on_wait) if (si is not None and si.on_wait) else []
                if len(waits) > max_waits:
                    for w in waits[: len(waits) - max_waits]:
                        nop = mybir.InstNoOp(name=f"wsplit_nop_{ctr}", ins=[], outs=[])
                        ctr += 1
                        nop.engine = inst.engine
                        nop.sync_info = mybir.SyncInfo(on_wait=[w], on_update=[])
                        new.append(nop)
                    si.on_wait = waits[len(waits) - max_waits:]
                new.append(inst)
            bb.instructions = new


def _patch_sin2pi():
    """sin2pi (ACT func id 99) is not in the concourse enum but lives in the
    exp_and_friends table set. Emit Arctan as a marker and rewrite the
    serialized BIR."""
    if getattr(bass.Bass, "_sin2pi_patched", False):
        return
    orig = bass.Bass.to_json_bytes

    def to_json_bytes(self, *a, **k):
        return orig(self, *a, **k).replace(b'"Arctan"', b'"Sin2pi"')

    bass.Bass.to_json_bytes = to_json_bytes
    bass.Bass._sin2pi_patched = True


def _patch_act_tables():
    """The sin2pi marker (Arctan) must look exp-table-compatible to the
    CoreSim cost model that DRIVES THE TILE SCHEDULER; otherwise every
    sin<->exp alternation is scheduled around a phantom ~1.4us ACT table
    reload and the semaphore schedule forces the hardware to follow that
    pessimistic timeline (the real Sin2pi shares exp_and_friends with Exp
    on hardware, so no reload ever happens).  get_activation_tables is
    functools.cache'd, so mutating the returned sets is sticky."""
    from concourse.hw_specs import get_activation_tables

    for arch in ("gen3",):
        try:
            tables = get_activation_tables(arch)
        except Exception:
            continue
        for funcs in tables.values():
            if mybir.ActivationFunctionType.Exp in funcs:
                funcs.add(mybir.ActivationFunctionType.Arctan)


_patch_tile_drain()
_patch_sin2pi()
_patch_act_tables()

SIN2PI = mybir.ActivationFunctionType.Arctan  # rewritten to Sin2pi in the BIR

# ---------------------------------------------------------------- row maps
# Feature row k in [0, 108): k = dout*36 + din*6 + f.
_K = np.arange(108)
_DOUT = _K // 36
_DIN = (_K % 36) // 6
_F = _K % 6

# x75 row layout:
#   rows  0..2 : thi0..2 (fp16 turns, for the cluster scores -- matmul
#                operands must start at partition 0)
#   rows  3..38: fp16 centered sin residue (2^f t_din) mod 1, idx din*6+f
#   rows 39..74: fp16 centered cos residue (2^f t_din + .25) mod 1
#                (pre-shifting keeps BOTH sin2pi arg halves in [-.5, .5],
#                the table's strict domain)


def _build_static_arrays(linear_mappings, centroids):
    L = np.asarray(linear_mappings, dtype=np.float32)       # [768, 72]
    cent = np.asarray(centroids, dtype=np.float32)          # [256, 3]

    # Angle gather (residues are pre-reduced AND pre-shifted on the host,
    # so both weight sets are pure 0/1 row-gathers). Column 108: zero in
    # both halves; the cos half's sin2pi gets bias .25 on row 108, making
    # it the denominator's 1.
    wea = np.zeros((XR, 109), dtype=np.float16)
    wea[3 + _DIN * 6 + _F, _K] = 1.0
    web = np.zeros((XR, 109), dtype=np.float16)
    web[39 + _DIN * 6 + _F, _K] = 1.0

    # Scores: score_c = sum_d x_d * cent[c, d];  x = 2 pi t
    cent2pi = (cent * _TWO_PI).astype(np.float16)           # [256, 3]
    c6 = np.ascontiguousarray(cent2pi.T)                    # [3, 256]

    # Per-partition bias for the single [109, 2NF] sin2pi: row 108 gets
    # .25 so the (zero) angle there becomes sin2pi(.25) = 1. The sin
    # half's row 108 also sees the bias, but encA row 108 is never read.
    qb = np.zeros((109, 1), dtype=np.float32)
    qb[108, 0] = 0.25

    # Lhat: G_A[k] pairs with sin rows, G_B with cos rows, col 108 = denom.
    lhata = np.zeros((256, 108), dtype=np.float32)
    lhatb = np.zeros((256, 109), dtype=np.float32)
    ecol_sin = _DIN * 12 + _F
    ecol_cos = _DIN * 12 + 6 + _F
    for k in range(108):
        lhata[:, k] = L[3 * np.arange(C) + _DOUT[k], ecol_sin[k]]
        lhatb[:, k] = L[3 * np.arange(C) + _DOUT[k], ecol_cos[k]]
    lhatb[:, 108] = 1.0

    lp = np.zeros((128, 434), dtype=NP_BF16)
    lp[:, 0:108] = lhata[0:128]
    lp[:, 108:216] = lhata[128:256]
    lp[:, 216:325] = lhatb[0:128]
    lp[:, 325:434] = lhatb[128:256]

    # Escape-group reduction weights: chunk m of a group lands on output
    # rows 4m..4m+3 of the shared [64, NF] PSUM accumulator (matmul output
    # base partition must be 0, so the row placement is done by shifting
    # the weight columns; all other rows get zeros, which the PSUM
    # accumulation chain leaves untouched).
    rpg = np.zeros((109, 128 * 16), dtype=NP_BF16)
    for m in range(16):
        base = 128 * m
        rpg[_K, base + 4 * m + _DOUT] = 1.0           # RA block
        rpg[_K, base + 64 + 4 * m + _DOUT] = 1.0      # RB block
        rpg[108, base + 64 + 4 * m + 3] = 1.0         # denominator row

    return wea, web, c6, qb, lp, rpg


def _build_x75(X):
    """Per-core [75, NPAD] fp16 slabs: centered per-(dim, freq) sin and
    cos residues plus raw fp16 turns for the scores."""
    t = (np.asarray(X, dtype=np.float64) / _TWO_PI)          # [N, 6]
    x75 = np.zeros((NCORES, XR, NPAD), dtype=np.float16)
    r = t[None, :, :] * (2.0 ** np.arange(NFREQ))[:, None, None]  # [F, N, 6]
    rs = r - np.round(r)                                     # [-0.5, 0.5]
    rc = (r + 0.25) - np.round(r + 0.25)
    thi = t.astype(np.float16)
    for c in range(NCORES):
        seg = slice(c * NPC, (c + 1) * NPC)
        for d in range(3):
            x75[c, d, :NPC] = thi[seg, d]
        for d in range(D):
            for f in range(NFREQ):
                x75[c, 3 + d * 6 + f, :NPC] = rs[f, seg, d]
                x75[c, 39 + d * 6 + f, :NPC] = rc[f, seg, d]
    return x75


def _build_program():
    nc = bass.Bass()
    x75_h = nc.dram_tensor("x75", [XR, NPAD], F16, kind="ExternalInput")
    wea_h = nc.dram_tensor("wea", [XR, 109], F16, kind="ExternalInput")
    web_h = nc.dram_tensor("web", [XR, 109], F16, kind="ExternalInput")
    c6_h = nc.dram_tensor("c6", [3, 256], F16, kind="ExternalInput")
    qb_h = nc.dram_tensor("qb", [109, 1], F32, kind="ExternalInput")
    lp_h = nc.dram_tensor("lp", [128, 434], BF16, kind="ExternalInput")
    rp_h = nc.dram_tensor("rp", [109, 128 * GRP], BF16, kind="ExternalInput")
    # Row 4*(chunk%GRP)+d, col (group*NF + point) holds channel d (0..2 rgb
    # numerators, 3 = softmax denominator) of that chunk's point.
    o4_h = nc.dram_tensor("o4", [4 * GRP, NG * NF], F32, kind="ExternalOutput")

    EXP = mybir.ActivationFunctionType.Exp
    SKIP = set(_os.environ.get("KB_SKIP", "").split(",")) - {""}

    with TileContext(nc) as tc:
        with (
            tc.tile_pool(name="statics", bufs=1) as statics,
            tc.tile_pool(name="enc", bufs=int(_os.environ.get("KB_E", 3))) as encpool,
            tc.tile_pool(name="esc", bufs=int(_os.environ.get("KB_S", 3))) as escpool,
            tc.tile_pool(name="pprod", bufs=int(_os.environ.get("KB_P", 3))) as ppool,
            tc.tile_pool(name="rgbs", bufs=int(_os.environ.get("KB_R", 2))) as rspool,
            # PSUM bank budget (8 banks x 2KB/partition):
            #   ang 2 + sc 2 + gA 1 + gB 1 + rgbd 1 + warm 1 = 8
            tc.tile_pool(name="ang", bufs=int(_os.environ.get("KB_ANG", 1)), space="PSUM") as angpool,
            tc.tile_pool(name="sc", bufs=int(_os.environ.get("KB_SC", 1)), space="PSUM") as scpool,
            tc.tile_pool(name="gA", bufs=int(_os.environ.get("KB_GA", 1)), space="PSUM") as gApool,
            tc.tile_pool(name="gB", bufs=int(_os.environ.get("KB_GB", 1)), space="PSUM") as gBpool,
            tc.tile_pool(name="rgbd", bufs=int(_os.environ.get("KB_RG", 1)), space="PSUM") as rgbdpool,
            tc.tile_pool(name="warm", bufs=1, space="PSUM") as warmpool,
        ):
            # The whole input slab is SBUF-resident: 75 partitions x
            # 50KB/partition.  It streams in at startup as a handful of
            # block DMAs (geometrically ramped so chunk 0 lands fast),
            # round-robined across the three DGE queues; the steady-state
            # loop then has NO per-chunk input DMA, no buffer recycling
            # WARs, and only one coarse slab-block wait per 4-8 chunks.
            blk_chunks = [1, 1, 2, 4, 4, 8, 8, 8, 8, 5]
            assert sum(blk_chunks) == CH
            blk_tiles = []
            chunk2blk = []
            c0 = 0
            dmaq = [nc.sync, nc.gpsimd]
            for bi, bc in enumerate(blk_chunks):
                bt = statics.tile([XR, bc * NF], F16, name=f"xblk{bi}")
                blk_tiles.append(bt)
                for j in range(bc):
                    chunk2blk.append((bi, j * NF))
                dmaq[bi % 2].dma_start(
                    out=bt[:], in_=x75_h[:, c0 * NF:(c0 + bc) * NF]
                )
                c0 += bc

            wea_t = statics.tile([XR, 109], F16)
            web_t = statics.tile([XR, 109], F16)
            c6_t = statics.tile([3, 256], F16)
            qb_t = statics.tile([109, 1], F32)
            lp_t = statics.tile([128, 434], BF16)
            # rp is 446KB -- by far the biggest static.  It goes through a
            # single ~22 GB/s DMA engine (~20us), and late(0) only needs
            # its first column group, so it is split into four separately
            # tracked tiles, ordered so early groups land first.  (A single
            # tile written by four DMAs would make late(0) wait for all
            # four -- tile dependency tracking is per-tile.)
            rp_ts = [statics.tile([109, 128 * 4], BF16, name=f"rp{j}")
                     for j in range(4)]
            # Statics ride the scalar HWDGE queue (one-time cost) so the
            # SP and gpsimd queues are clear for the input stream from the
            # first chunk on.  Order: everything the first chunks need
            # comes before the rp tail.
            nc.scalar.dma_start(out=wea_t[:], in_=wea_h[:])
            nc.scalar.dma_start(out=web_t[:], in_=web_h[:])
            nc.scalar.dma_start(out=c6_t[:], in_=c6_h[:])
            nc.scalar.dma_start(out=qb_t[:], in_=qb_h[:])
            nc.scalar.dma_start(out=lp_t[:], in_=lp_h[:])
            for j in range(4):
                nc.scalar.dma_start(
                    out=rp_ts[j][:], in_=rp_h[:, 512 * j:512 * (j + 1)]
                )

            # 1x1 scratch "tick" copies: cheap sem-incrementing ops the
            # scheduler can place freely; the ACT tick keeps the single-
            # buffered score bank from chaining the next chunk behind
            # sin2pi (pinning ticks to producer outputs measures worse --
            # the scheduler places free ticks better).
            dve_tk = statics.tile([1, 2], F16)
            act_tk = statics.tile([1, 2], F16)
            nc.vector.memset(dve_tk[:], 0.0)
            nc.vector.memset(act_tk[:], 0.0)

            def dve_tick():
                nc.vector.tensor_copy(out=dve_tk[0:1, 1:2], in_=dve_tk[0:1, 0:1])

            def act_tick():
                nc.scalar.copy(out=act_tk[0:1, 1:2], in_=act_tk[0:1, 0:1])

            # Dummy 1x1 exp: pre-loads the exp_and_friends ACT table during
            # the input-DMA ramp (it only depends on the memset scratch), so
            # the first real activation doesn't pay the ~1.3 us table load.
            nc.scalar.activation(
                out=act_tk[0:1, 1:2], in_=act_tk[0:1, 0:1],
                func=EXP, bias=0.0, scale=1.0,
            )

            # Dummy matmuls on scratch: pre-warm the PE p-state during the
            # input-DMA ramp so the first real matmuls run at full clock.
            p_st = statics.tile([109, 2 * NF], BF16)
            enc_st = statics.tile([109, 2 * NF], F16)
            esc_st = statics.tile([128, 2 * NF], BF16)
            nc.vector.memset(p_st[:], 0.01)
            nc.vector.memset(enc_st[:], 0.01)
            nc.vector.memset(esc_st[:], 0.01)

            warm_x = statics.tile([1, NF], F16)
            nc.vector.memset(warm_x[:], 0.0)
            warm_o = warmpool.tile([1, NF], F32)
            for _w in range(2):
                nc.tensor.matmul(
                    warm_o[0:1, 0:32], warm_x[0:1, 0:1], warm_x[0:1, 0:32],
                    start=True, stop=True,
                )

            # Software-pipelined: iteration k issues the early stages of
            # chunk k, the mid stages (G, P-mult) of chunk k-1, and the late
            # stages (reduction, escape) of chunk k-2.  Per-engine program
            # order then never forces a cheap early op of chunk k to queue
            # behind an expensive late-chain op of the same chunk, which
            # would serialize the whole loop at dependency-cycle latency.
            # The Tile wait machinery makes each cross-engine consumer wait
            # for completion of the instruction FOLLOWING its true producer
            # on the producer's engine.  The engine nops after each critical
            # producer make that "+1" instruction free, so consumers fire at
            # producer-completion instead of queueing behind the producer
            # engine's next (possibly late-blocking) real op.
            def early_sc(i):
                bi, off = chunk2blk[i]
                xt = blk_tiles[bi][:, off:off + NF]

                sc = scpool.tile([128, 2 * NF], F32)
                nc.tensor.matmul(
                    sc[:, 0:NF], c6_t[:, 0:128], xt[0:3, :],
                    start=True, stop=True,
                )
                nc.tensor.matmul(
                    sc[:, NF:2 * NF], c6_t[:, 128:256], xt[0:3, :],
                    start=True, stop=True,
                )
                return xt, sc

            def mid(st):
                enc, esc = st
                if "act" in SKIP:
                    enc, esc = enc_st, esc_st
                gA = gApool.tile([108, NF], F32)
                gB = gBpool.tile([109, NF], F32)
                p = ppool.tile([109, 2 * NF], BF16)
                nc.tensor.matmul(
                    gA[:], lp_t[:, 0:108], esc[:, 0:NF],
                    start=True, stop=False,
                )
                nc.tensor.matmul(
                    gA[:], lp_t[:, 108:216], esc[:, NF:2 * NF],
                    start=False, stop=True,
                )
                if "pm" not in SKIP:
                    nc.vector.tensor_mul(p[0:108, 0:NF], gA[:], enc[0:108, 0:NF])
                    dve_tick()
                nc.tensor.matmul(
                    gB[:], lp_t[:, 216:325], esc[:, 0:NF],
                    start=True, stop=False,
                )
                nc.tensor.matmul(
                    gB[:], lp_t[:, 325:434], esc[:, NF:2 * NF],
                    start=False, stop=True,
                )
                if "pm" not in SKIP:
                    nc.vector.tensor_mul(p[:, NF:2 * NF], gB[:], enc[:, NF:2 * NF])
                    dve_tick()
                else:
                    p = p_st
                return p

            def early_act(i, xt):
                # Angles land already range-reduced (host residues): sin
                # half in [-.51, .51], cos half (pre-shifted residues) too.
                # Emitted AFTER the previous chunk's g matmuls so the DVE
                # mults start as early as possible, and right BEFORE the
                # ACT ops so sin2pi's wait resolves to the exact angB end.
                ang = angpool.tile([109, 2 * NF], F32)
                nc.tensor.matmul(
                    ang[0:109, 0:NF], wea_t[:], xt[:], start=True, stop=True
                )
                nc.tensor.matmul(
                    ang[0:109, NF:2 * NF], web_t[:], xt[:],
                    start=True, stop=True,
                )
                return ang

            def early_fin(i, sc, ang):
                if "act" in SKIP:
                    return enc_st, esc_st
                # sin2pi first: the ang-bank read completes early in the
                # iteration, so the single-wait assigner can give the next
                # chunk's ang matmul a cheap dominating wait.  Row 108 of
                # the per-partition bias turns the cos half's zero angle
                # into sin2pi(.25) = 1, the softmax denominator.
                enc = encpool.tile([109, 2 * NF], F16)
                nc.scalar.activation(
                    out=enc[:], in_=ang[:], func=SIN2PI,
                    bias=qb_t[:], scale=1.0,
                )
                act_tick()
                esc = escpool.tile([128, 2 * NF], BF16)
                nc.scalar.activation(
                    out=esc[:], in_=sc[:], func=EXP, bias=0.0, scale=1.0
                )
                act_tick()
                return enc, esc

            rgbdg = [None]   # current [64, NF] escape-group accumulator

            def late(p, i):
                m = i % GRP
                j = i // GRP
                if m == 0:
                    rgbdg[0] = rgbdpool.tile([4 * GRP, NF], F32, name="rgbdg")
                rgbd = rgbdg[0]
                last = (m == GRP - 1) or (i == CH - 1)
                rp_t = rp_ts[m // 4]
                mm = m % 4
                nc.tensor.matmul(
                    rgbd[:],
                    rp_t[0:108, 128 * mm:128 * mm + 64], p[0:108, 0:NF],
                    start=(m == 0), stop=False,
                )
                nc.tensor.matmul(
                    rgbd[:],
                    rp_t[0:109, 128 * mm + 64:128 * mm + 128],
                    p[0:109, NF:2 * NF],
                    start=False, stop=last,
                )

                # One escape per GRP chunks: the copy's fixed cost is
                # amortized 16x (cost scales with columns, not partitions).
                # It rides on DVE, the least-loaded PSUM-capable engine;
                # the DMA rides gpsimd (tiny next to the input stream).
                if m == GRP - 1 or i == CH - 1:
                    rows = 4 * (m + 1)
                    rgbs = rspool.tile([4 * GRP, NF], F32)
                    nc.vector.tensor_copy(out=rgbs[0:rows, :], in_=rgbd[0:rows, :])
                    dve_tick()
                    nc.gpsimd.dma_start(
                        out=o4_h[0:rows, j * NF:(j + 1) * NF],
                        in_=rgbs[0:rows, :],
                    )

            prev_e = None   # (enc, esc) of chunk i-1
            prev_m = None   # p of chunk i-2
            prev_m2 = None  # p of chunk i-3
            for i in range(CH + 3):
                # late() leads: the reduction matmuls' inputs are THREE
                # chunks old, so at the head of the PE stream they never
                # wait and the engine stays dense.  (At depth two the DVE
                # products pA/pB -- which can only start after the G
                # matmuls that run at the END of the previous PE cycle --
                # arrive ~0.7 us late and stall both late matmuls every
                # chunk, which also keeps resetting the PE clock ramp.)
                if prev_m2 is not None:
                    late(prev_m2, i - 3)
                xs = early_sc(i) if i < CH else None
                m = mid(prev_e) if prev_e is not None else None
                if xs is not None:
                    ang = early_act(i, xs[0])
                    e = early_fin(i, xs[1], ang)
                else:
                    e = None
                prev_e, prev_m2, prev_m = e, prev_m, m

    _split_excess_waits(nc)
    return nc


_PROGRAM = None


def _get_program():
    global _PROGRAM
    if _PROGRAM is None:
        _PROGRAM = _build_program()
    return _PROGRAM


def kernel(X, linear_mappings, centroids, _want_trace=False):
    wea, web, c6, qb, lp, rp = _build_static_arrays(linear_mappings, centroids)
    x75 = _build_x75(X)

    nc = _get_program()
    in_maps = [
        {
            "x75": np.ascontiguousarray(x75[c]),
            "wea": wea, "web": web, "c6": c6, "qb": qb, "lp": lp, "rp": rp,
        }
        for c in range(NCORES)
    ]
    res = run_bass_kernel_spmd(
        nc, in_maps, core_ids=list(range(NCORES)), trace=_want_trace
    )

    out = np.empty((N, 3), dtype=np.float32)
    for c in range(NCORES):
        o4 = res.results[c]["o4"]                  # [4*GRP, NG*NF] f32
        # row 4*(chunk%GRP)+d, col (chunk//GRP)*NF + p -> chunk point p
        o4c = o4.reshape(GRP, 4, NG, NF)           # [m, d, j, p]
        o4c = o4c.transpose(2, 0, 3, 1)            # [j, m, p, d]
        flat = o4c.reshape(NG * GRP * NF, 4)[:NPC]  # per-core points
        out[c * NPC:(c + 1) * NPC, :] = flat[:, 0:3] / flat[:, 3:4]
    if _want_trace:
        return out, res
    return out
